# revision 8
# baseline (speedup 1.0000x reference)
"""Contrastive-loss kernel for Trainium2, 8 NeuronCores (SPMD data-parallel).

Math (matches the reference):
  xn = x / ||x||, pn = positive / ||positive||, nn = negative / ||negative||
  sim_b    = <xn_b, pn_b>
  denom_b  = sum_c exp(<xn_b, nn_c>)
  loss     = -sum_b (sim_b - log denom_b) / bn

Sharding: data-parallel on bn. Core i gets rows [i*1024, (i+1)*1024) of
x/positive plus the full (replicated) negative. Each core outputs its
per-row (sim_b - log denom_b) values; the host sums and scales.

Per-core pipeline (PE roofline ~220us: 1024 bf16 matmuls of [128x128]@[128x512]):
  - phase A: x/positive shard stats via DVE scalar_tensor_tensor (one-pass
    square+reduce); rsqrt on DVE (bit-trick + 3 Newton steps — keeps ACT
    free for Exp, avoiding table switches); normalize+cast x to bf16;
    round-trip via internal DRAM + DMA xbar transpose to d-major xnT.
  - negatives in 8 super-chunks of 2048 rows, software-pipelined: phase B
    of chunk sc+1 is emitted before phase C of chunk sc so DVE/DMA prep of
    the next chunk overlaps PE matmuls of the current one (keeps HAM warm).
    B: norms (DVE), rsqrt (DVE), normalize+cast bf16, k-major DRAM write,
    xbar-transpose read to [128, 2048] d-major tiles.
    C: 4x4 bf16 matmuls accumulate into [128, 2048] PSUM (4 banks); ACT Exp
    in place with accum_out = per-row partial denominator.
  - finish: reduce partials, ACT Ln (single table switch), subtract sim,
    DMA out [128, 8] per-row values.
"""

import numpy as np
from contextlib import ExitStack

import concourse.bass as bass
import concourse.tile as tile
from concourse import bacc, mybir
from concourse.bass_utils import run_bass_kernel_spmd

NCORES = 8
BN, CN, D = 8192, 16384, 512
P = 128
BN_LOC = BN // NCORES           # 1024 rows of x per core
C_SUPER = 2048                  # negatives per super-chunk (4 PSUM banks)

F32 = mybir.dt.float32
I32 = mybir.dt.int32
BF16 = mybir.dt.bfloat16
EXP = mybir.ActivationFunctionType.Exp
LN = mybir.ActivationFunctionType.Ln
MULT = mybir.AluOpType.mult
ADD = mybir.AluOpType.add
SUB = mybir.AluOpType.subtract
SHR = mybir.AluOpType.logical_shift_right


def _rsqrt_dve(nc, pool, out, in_, shape):
    """out = 1/sqrt(in_) on the vector engine: bit-trick seed + 3 Newton steps.

    Rel err ~2e-7 (hardware-verified). Avoids ScalarE table switches so the
    activation engine runs Exp only during the main loop.
    """
    yi = pool.tile(shape, I32, tag="rsq_i")
    nc.vector.tensor_scalar(out=yi[:], in0=in_.bitcast(I32), scalar1=1,
                            scalar2=None, op0=SHR)
    nc.vector.tensor_scalar(out=yi[:], in0=yi[:], scalar1=0x5F3759DF,
                            scalar2=-1, op0=SUB, op1=MULT)
    y = yi[:].bitcast(F32)
    h = pool.tile(shape, F32, tag="rsq_h")
    nc.vector.tensor_scalar_mul(h[:], in_, -0.5)
    for _ in range(3):
        t2 = pool.tile(shape, F32, tag="rsq_t")
        nc.vector.tensor_mul(t2[:], y, y)
        nc.vector.tensor_mul(t2[:], t2[:], h[:])
        nc.vector.tensor_scalar_add(t2[:], t2[:], 1.5)
        nc.vector.tensor_mul(y, y, t2[:])
    nc.vector.tensor_copy(out=out, in_=y)


def _build_kernel(ctx: ExitStack, tc: tile.TileContext, x_ap, p_ap, n_ap, out_ap,
                  bn_loc=BN_LOC, cn=CN, d=D, c_super=C_SUPER):
    nc = tc.nc
    B_TILES = bn_loc // P       # b-tiles of 128 rows
    KT = d // P                 # contraction k-tiles
    N_SC = cn // c_super        # super-chunks of negatives
    NEG_T_SC = c_super // P     # [128, d] negative tiles per super-chunk
    NSUB = c_super // 512       # matmul column sub-tiles (one PSUM bank each)

    big = ctx.enter_context(tc.tile_pool(name="big", bufs=1))
    stats = ctx.enter_context(tc.tile_pool(name="stats", bufs=1))
    statsd = ctx.enter_context(tc.tile_pool(name="statsd", bufs=2))
    rsqp = ctx.enter_context(tc.tile_pool(name="rsqp", bufs=2))
    negp = ctx.enter_context(tc.tile_pool(name="negp", bufs=20))
    bf16p = ctx.enter_context(tc.tile_pool(name="bf16p", bufs=4))
    sqp = ctx.enter_context(tc.tile_pool(name="sqp", bufs=3))
    nntp = ctx.enter_context(tc.tile_pool(name="nntp", bufs=8))
    psump = ctx.enter_context(tc.tile_pool(name="psump", bufs=2, space="PSUM"))
    dram1 = ctx.enter_context(tc.tile_pool(name="dram1", bufs=1, space="DRAM"))
    dram2 = ctx.enter_context(tc.tile_pool(name="dram2", bufs=3, space="DRAM"))

    # ---- Phase A: x / positive shard: norms, sim, normalized bf16 xnT ----
    x_all = big.tile([P, B_TILES, d], F32)
    nc.gpsimd.dma_start(x_all[:], x_ap.rearrange("(t p) d -> p t d", p=P))
    p_all = big.tile([P, B_TILES, d], F32)
    nc.gpsimd.dma_start(p_all[:], p_ap.rearrange("(t p) d -> p t d", p=P))

    xss = stats.tile([P, B_TILES], F32)
    pss = stats.tile([P, B_TILES], F32)
    xps = stats.tile([P, B_TILES], F32)
    for b in range(B_TILES):
        for src0, src1, dst in ((x_all, x_all, xss), (p_all, p_all, pss), (x_all, p_all, xps)):
            sq = sqp.tile([P, d], F32, tag="sq")
            nc.vector.scalar_tensor_tensor(
                out=sq[:], in0=src0[:, b], scalar=1.0, in1=src1[:, b],
                op0=MULT, op1=MULT, accum_out=dst[:, b : b + 1],
            )

    inv_x = stats.tile([P, B_TILES], F32)
    _rsqrt_dve(nc, rsqp, inv_x[:], xss[:], [P, B_TILES])
    inv_p = stats.tile([P, B_TILES], F32)
    _rsqrt_dve(nc, rsqp, inv_p[:], pss[:], [P, B_TILES])

    sim = stats.tile([P, B_TILES], F32)
    nc.vector.tensor_tensor(out=sim[:], in0=xps[:], in1=inv_x[:], op=MULT)
    nc.vector.tensor_tensor(out=sim[:], in0=sim[:], in1=inv_p[:], op=MULT)

    # normalize + cast x to bf16, store k-major so the xbar transpose read
    # is contiguous: xnb[k, row, c] = xn[row, k*128 + c]
    xnb = dram1.tile([KT, bn_loc, P], BF16, tag="xnb")
    for b in range(B_TILES):
        xn_t = bf16p.tile([P, d], BF16, tag="xn_t")
        nc.vector.tensor_scalar_mul(xn_t[:], x_all[:, b], inv_x[:, b : b + 1])
        nc.gpsimd.dma_start(
            xnb[:, b * P : (b + 1) * P, :].rearrange("k p c -> p k c"),
            xn_t[:].rearrange("p (k c) -> p k c", k=KT),
        )

    xnT = big.tile([P, KT, bn_loc], BF16)  # xnT[dp, k, row] = xn[row, k*128+dp]
    for k in range(KT):
        nc.sync.dma_start_transpose(xnT[:, k, :], xnb[k])

    # ---- Phases B+C over negative super-chunks, software-pipelined ----
    dparts = stats.tile([P, B_TILES, N_SC], F32)
    nnT_of_sc = {}

    def phase_b(sc):
        """Load+normalize 2048 negatives, write k-major bf16, transpose back."""
        nss = statsd.tile([P, NEG_T_SC], F32, tag="nss")
        neg_ts = []
        for j in range(NEG_T_SC):
            r0 = (sc * NEG_T_SC + j) * P
            neg_t = negp.tile([P, d], F32, tag="neg")
            nc.gpsimd.dma_start(neg_t[:], n_ap[r0 : r0 + P, :])
            sq = sqp.tile([P, d], F32, tag="sq")
            nc.vector.scalar_tensor_tensor(
                out=sq[:], in0=neg_t[:], scalar=1.0, in1=neg_t[:],
                op0=MULT, op1=MULT, accum_out=nss[:, j : j + 1],
            )
            neg_ts.append(neg_t)
        inv_n = statsd.tile([P, NEG_T_SC], F32, tag="invn")
        _rsqrt_dve(nc, rsqp, inv_n[:], nss[:], [P, NEG_T_SC])

        nnb = dram2.tile([KT, c_super, P], BF16, tag="nnb")
        for j in range(NEG_T_SC):
            nnb_t = bf16p.tile([P, d], BF16, tag="nnb_t")
            nc.vector.tensor_scalar_mul(nnb_t[:], neg_ts[j][:], inv_n[:, j : j + 1])
            nc.gpsimd.dma_start(
                nnb[:, j * P : (j + 1) * P, :].rearrange("k p c -> p k c"),
                nnb_t[:].rearrange("p (k c) -> p k c", k=KT),
            )

        nnT = []
        for k in range(KT):
            t = nntp.tile([P, c_super], BF16, tag="nnT")
            nc.sync.dma_start_transpose(t[:], nnb[k])
            nnT.append(t)
        nnT_of_sc[sc] = nnT

    def phase_c(sc):
        """Matmuls for all b-tiles against this chunk + in-place exp/accum."""
        nnT = nnT_of_sc.pop(sc)
        for b in range(B_TILES):
            ps = psump.tile([P, c_super], F32, tag="ps")
            for k in range(KT):
                for sub in range(NSUB):
                    nc.tensor.matmul(
                        ps[:, sub * 512 : (sub + 1) * 512],
                        xnT[:, k, b * P : (b + 1) * P],
                        nnT[k][:, sub * 512 : (sub + 1) * 512],
                        start=(k == 0),
                        stop=(k == KT - 1),
                    )
            nc.scalar.activation(
                out=ps[:], in_=ps[:], func=EXP,
                accum_out=dparts[:, b, sc : sc + 1],
            )

    phase_b(0)
    for sc in range(N_SC):
        if sc + 1 < N_SC:
            phase_b(sc + 1)
        phase_c(sc)

    # ---- Phase D: denom -> log -> rowloss ----
    denoms = stats.tile([P, B_TILES], F32)
    for b in range(B_TILES):
        nc.vector.tensor_reduce(
            denoms[:, b : b + 1], dparts[:, b, :],
            axis=mybir.AxisListType.X, op=ADD,
        )
    lnden = stats.tile([P, B_TILES], F32)
    nc.scalar.activation(out=lnden[:], in_=denoms[:], func=LN)
    rowloss = stats.tile([P, B_TILES], F32)
    nc.vector.tensor_sub(out=rowloss[:], in0=sim[:], in1=lnden[:])
    nc.gpsimd.dma_start(out_ap[:], rowloss[:])


_CACHED_NC = None


def _get_program():
    global _CACHED_NC
    if _CACHED_NC is None:
        nc = bacc.Bacc(
            "TRN2", target_bir_lowering=False, debug=False, num_devices=NCORES
        )
        x_ap = nc.dram_tensor("x_shard", [BN_LOC, D], F32, kind="ExternalInput").ap()
        p_ap = nc.dram_tensor("pos_shard", [BN_LOC, D], F32, kind="ExternalInput").ap()
        n_ap = nc.dram_tensor("negative", [CN, D], F32, kind="ExternalInput").ap()
        out_ap = nc.dram_tensor("rowloss", [P, BN_LOC // P], F32, kind="ExternalOutput").ap()
        with tile.TileContext(nc) as tc, ExitStack() as ctx:
            _build_kernel(ctx, tc, x_ap, p_ap, n_ap, out_ap)
        nc.compile()
        _CACHED_NC = nc
    return _CACHED_NC


def run_sharded(x, positive, negative, **spmd_kwargs):
    """Run the SPMD program; returns BassKernelResults."""
    nc = _get_program()
    x = np.ascontiguousarray(x, dtype=np.float32)
    positive = np.ascontiguousarray(positive, dtype=np.float32)
    negative = np.ascontiguousarray(negative, dtype=np.float32)
    in_maps = [
        {
            "x_shard": x[i * BN_LOC : (i + 1) * BN_LOC],
            "pos_shard": positive[i * BN_LOC : (i + 1) * BN_LOC],
            "negative": negative,
        }
        for i in range(NCORES)
    ]
    res = run_bass_kernel_spmd(nc, in_maps, list(range(NCORES)), **spmd_kwargs)
    return res


def kernel(x, positive, negative):
    res = run_sharded(x, positive, negative)
    total = 0.0
    for i in range(NCORES):
        total += res.results[i]["rowloss"].astype(np.float64).sum()
    return np.asarray(-(total / BN), dtype=np.float32)


# revision 12
# speedup vs baseline: 1.1002x; 1.1002x over previous
"""Contrastive-loss kernel for Trainium2, 8 NeuronCores (SPMD data-parallel).

Math (matches the reference):
  xn = x / ||x||, pn = positive / ||positive||, nn = negative / ||negative||
  sim_b    = <xn_b, pn_b>
  denom_b  = sum_c exp(<xn_b, nn_c>)
  loss     = -sum_b (sim_b - log denom_b) / bn

Sharding: data-parallel on bn. Core i gets rows [i*1024, (i+1)*1024) of
x/positive plus the full (replicated) negative. Each core outputs its
per-row (sim_b - log denom_b) values; the host sums and scales.

Per-core pipeline (PE roofline ~220us: 1024 bf16 matmuls of [128x128]@[128x512]):
  - phase A: x/positive shard stats via DVE scalar_tensor_tensor (one-pass
    square+reduce); rsqrt on DVE (bit-trick + 3 Newton steps — keeps ACT
    free for Exp, avoiding table switches); normalize+cast x to bf16;
    round-trip via internal DRAM + DMA xbar transpose to d-major xnT.
  - negatives in 8 super-chunks of 2048 rows, software-pipelined: phase B
    of chunk sc+1 is emitted before phase C of chunk sc so DVE/DMA prep of
    the next chunk overlaps PE matmuls of the current one (keeps HAM warm).
    B: norms (DVE), rsqrt (DVE), normalize+cast bf16, k-major DRAM write,
    xbar-transpose read to [128, 2048] d-major tiles.
    C: 4x4 bf16 matmuls accumulate into [128, 2048] PSUM (4 banks); ACT Exp
    in place with accum_out = per-row partial denominator.
  - finish: reduce partials, ACT Ln (single table switch), subtract sim,
    DMA out [128, 8] per-row values.
"""

import numpy as np
from contextlib import ExitStack

import concourse.bass as bass
import concourse.tile as tile
from concourse import bacc, mybir
from concourse.bass_utils import run_bass_kernel_spmd

NCORES = 8
BN, CN, D = 8192, 16384, 512
P = 128
BN_LOC = BN // NCORES           # 1024 rows of x per core
C_SUPER = 2048                  # negatives per super-chunk (4 PSUM banks)

F32 = mybir.dt.float32
I32 = mybir.dt.int32
BF16 = mybir.dt.bfloat16
EXP = mybir.ActivationFunctionType.Exp
LN = mybir.ActivationFunctionType.Ln
MULT = mybir.AluOpType.mult
ADD = mybir.AluOpType.add
SUB = mybir.AluOpType.subtract
SHR = mybir.AluOpType.logical_shift_right


def _rsqrt_dve(nc, pool, out, in_, shape):
    """out = 1/sqrt(in_) on the vector engine: bit-trick seed + 3 Newton steps.

    Rel err ~2e-7 (hardware-verified). Avoids ScalarE table switches so the
    activation engine runs Exp only during the main loop.
    """
    yi = pool.tile(shape, I32, tag="rsq_i")
    nc.vector.tensor_scalar(out=yi[:], in0=in_.bitcast(I32), scalar1=1,
                            scalar2=None, op0=SHR)
    nc.vector.tensor_scalar(out=yi[:], in0=yi[:], scalar1=0x5F3759DF,
                            scalar2=-1, op0=SUB, op1=MULT)
    y = yi[:].bitcast(F32)
    h = pool.tile(shape, F32, tag="rsq_h")
    nc.vector.tensor_scalar_mul(h[:], in_, -0.5)
    for _ in range(3):
        t2 = pool.tile(shape, F32, tag="rsq_t")
        nc.vector.tensor_mul(t2[:], y, y)
        nc.vector.tensor_mul(t2[:], t2[:], h[:])
        nc.vector.tensor_scalar_add(t2[:], t2[:], 1.5)
        nc.vector.tensor_mul(y, y, t2[:])
    nc.vector.tensor_copy(out=out, in_=y)


def _build_kernel(ctx: ExitStack, tc: tile.TileContext, x_ap, p_ap, n_ap, out_ap,
                  bn_loc=BN_LOC, cn=CN, d=D, c_super=C_SUPER):
    nc = tc.nc
    B_TILES = bn_loc // P       # b-tiles of 128 rows
    KT = d // P                 # contraction k-tiles
    N_SC = cn // c_super        # super-chunks of negatives
    NEG_T_SC = c_super // P     # [128, d] negative tiles per super-chunk
    NSUB = c_super // 512       # matmul column sub-tiles (one PSUM bank each)

    big = ctx.enter_context(tc.tile_pool(name="big", bufs=1))
    stats = ctx.enter_context(tc.tile_pool(name="stats", bufs=1))
    statsd = ctx.enter_context(tc.tile_pool(name="statsd", bufs=3))
    rsqp = ctx.enter_context(tc.tile_pool(name="rsqp", bufs=2))
    negp = ctx.enter_context(tc.tile_pool(name="negp", bufs=36))
    bf16p = ctx.enter_context(tc.tile_pool(name="bf16p", bufs=4))
    sqp = ctx.enter_context(tc.tile_pool(name="sqp", bufs=3))
    nntp = ctx.enter_context(tc.tile_pool(name="nntp", bufs=12))
    psump = ctx.enter_context(tc.tile_pool(name="psump", bufs=2, space="PSUM"))
    dram1 = ctx.enter_context(tc.tile_pool(name="dram1", bufs=1, space="DRAM"))
    dram2 = ctx.enter_context(tc.tile_pool(name="dram2", bufs=3, space="DRAM"))

    # ---- Phase A (critical prefix only): xnT = transpose of RAW bf16 x.
    # x is NOT pre-normalized: 1/||x_b|| is a per-row (= per-PSUM-partition)
    # factor, folded into the Exp activation's scale AP later. This keeps
    # all row statistics off the xnT critical path.
    x_all = big.tile([P, B_TILES, d], F32)
    nc.sync.dma_start(x_all[:], x_ap.rearrange("(t p) d -> p t d", p=P))

    xss = stats.tile([P, B_TILES], F32)
    for b in range(B_TILES):
        sq = sqp.tile([P, d], F32, tag="sq")
        nc.vector.scalar_tensor_tensor(
            out=sq[:], in0=x_all[:, b], scalar=1.0, in1=x_all[:, b],
            op0=MULT, op1=MULT, accum_out=xss[:, b : b + 1],
        )
    inv_x = stats.tile([P, B_TILES], F32)
    _rsqrt_dve(nc, rsqp, inv_x[:], xss[:], [P, B_TILES])

    # cast raw x to bf16, store k-major so the xbar transpose read is
    # contiguous: xnb[k, row, c] = x_bf16[row, k*128 + c]
    xnb = dram1.tile([KT, bn_loc, P], BF16, tag="xnb")
    for b in range(B_TILES):
        xn_t = bf16p.tile([P, d], BF16, tag="xn_t")
        nc.vector.tensor_copy(out=xn_t[:], in_=x_all[:, b])
        nc.scalar.dma_start(
            xnb[:, b * P : (b + 1) * P, :].rearrange("k p c -> p k c"),
            xn_t[:].rearrange("p (k c) -> p k c", k=KT),
        )

    xnT = big.tile([P, KT, bn_loc], BF16)  # xnT[dp, k, row] = x_bf16[row, k*128+dp]
    for k in range(KT):
        nc.sync.dma_start_transpose(xnT[:, k, :], xnb[k])

    # ---- Phases B+C over negative super-chunks, software-pipelined ----
    dparts = stats.tile([P, B_TILES, N_SC], F32)
    nnT_of_sc = {}

    def phase_b(sc):
        """Load+normalize 2048 negatives, write k-major bf16, transpose back."""
        nss = statsd.tile([P, NEG_T_SC], F32, tag="nss")
        neg_ts = []
        for j in range(NEG_T_SC):
            r0 = (sc * NEG_T_SC + j) * P
            neg_t = negp.tile([P, d], F32, tag="neg")
            nc.gpsimd.dma_start(neg_t[:], n_ap[r0 : r0 + P, :])
            sq = sqp.tile([P, d], F32, tag="sq")
            nc.vector.scalar_tensor_tensor(
                out=sq[:], in0=neg_t[:], scalar=1.0, in1=neg_t[:],
                op0=MULT, op1=MULT, accum_out=nss[:, j : j + 1],
            )
            neg_ts.append(neg_t)
        inv_n = statsd.tile([P, NEG_T_SC], F32, tag="invn")
        _rsqrt_dve(nc, rsqp, inv_n[:], nss[:], [P, NEG_T_SC])

        nnb = dram2.tile([KT, c_super, P], BF16, tag="nnb")
        for j in range(NEG_T_SC):
            nnb_t = bf16p.tile([P, d], BF16, tag="nnb_t")
            nc.vector.tensor_scalar_mul(nnb_t[:], neg_ts[j][:], inv_n[:, j : j + 1])
            nc.gpsimd.dma_start(
                nnb[:, j * P : (j + 1) * P, :].rearrange("k p c -> p k c"),
                nnb_t[:].rearrange("p (k c) -> p k c", k=KT),
            )

        nnT = []
        for k in range(KT):
            t = nntp.tile([P, c_super], BF16, tag="nnT")
            nc.sync.dma_start_transpose(t[:], nnb[k])
            nnT.append(t)
        nnT_of_sc[sc] = nnT

    def phase_c(sc):
        """Matmuls for all b-tiles against this chunk + in-place exp/accum."""
        nnT = nnT_of_sc.pop(sc)
        for b in range(B_TILES):
            ps = psump.tile([P, c_super], F32, tag="ps")
            for k in range(KT):
                for sub in range(NSUB):
                    nc.tensor.matmul(
                        ps[:, sub * 512 : (sub + 1) * 512],
                        xnT[:, k, b * P : (b + 1) * P],
                        nnT[k][:, sub * 512 : (sub + 1) * 512],
                        start=(k == 0),
                        stop=(k == KT - 1),
                    )
            # exp(raw_logits / ||x_b||): per-row scale via the activation's
            # free affine (scale AP broadcasts per partition)
            nc.scalar.activation(
                out=ps[:], in_=ps[:], func=EXP,
                scale=inv_x[:, b : b + 1],
                accum_out=dparts[:, b, sc : sc + 1],
            )

    def phase_sim():
        """positive stats + sim (off the critical path)."""
        p_all = big.tile([P, B_TILES, d], F32)
        nc.sync.dma_start(p_all[:], p_ap.rearrange("(t p) d -> p t d", p=P))
        pss = stats.tile([P, B_TILES], F32)
        xps = stats.tile([P, B_TILES], F32)
        for b in range(B_TILES):
            for src0, src1, dst in ((p_all, p_all, pss), (x_all, p_all, xps)):
                sq = sqp.tile([P, d], F32, tag="sq")
                nc.vector.scalar_tensor_tensor(
                    out=sq[:], in0=src0[:, b], scalar=1.0, in1=src1[:, b],
                    op0=MULT, op1=MULT, accum_out=dst[:, b : b + 1],
                )
        inv_p = stats.tile([P, B_TILES], F32)
        _rsqrt_dve(nc, rsqp, inv_p[:], pss[:], [P, B_TILES])
        sim = stats.tile([P, B_TILES], F32)
        nc.vector.tensor_tensor(out=sim[:], in0=xps[:], in1=inv_x[:], op=MULT)
        nc.vector.tensor_tensor(out=sim[:], in0=sim[:], in1=inv_p[:], op=MULT)
        return sim

    for sc in range(min(2, N_SC)):
        phase_b(sc)
    sim = phase_sim()
    for sc in range(N_SC):
        if sc + 2 < N_SC:
            phase_b(sc + 2)
        phase_c(sc)

    # ---- Phase D: denom -> log -> rowloss ----
    denoms = stats.tile([P, B_TILES], F32)
    for b in range(B_TILES):
        nc.vector.tensor_reduce(
            denoms[:, b : b + 1], dparts[:, b, :],
            axis=mybir.AxisListType.X, op=ADD,
        )
    lnden = stats.tile([P, B_TILES], F32)
    nc.scalar.activation(out=lnden[:], in_=denoms[:], func=LN)
    rowloss = stats.tile([P, B_TILES], F32)
    nc.vector.tensor_sub(out=rowloss[:], in0=sim[:], in1=lnden[:])
    nc.gpsimd.dma_start(out_ap[:], rowloss[:])


_CACHED_NC = None


def _get_program():
    global _CACHED_NC
    if _CACHED_NC is None:
        nc = bacc.Bacc(
            "TRN2", target_bir_lowering=False, debug=False, num_devices=NCORES
        )
        x_ap = nc.dram_tensor("x_shard", [BN_LOC, D], F32, kind="ExternalInput").ap()
        p_ap = nc.dram_tensor("pos_shard", [BN_LOC, D], F32, kind="ExternalInput").ap()
        n_ap = nc.dram_tensor("negative", [CN, D], F32, kind="ExternalInput").ap()
        out_ap = nc.dram_tensor("rowloss", [P, BN_LOC // P], F32, kind="ExternalOutput").ap()
        with tile.TileContext(nc) as tc, ExitStack() as ctx:
            _build_kernel(ctx, tc, x_ap, p_ap, n_ap, out_ap)
        nc.compile()
        _CACHED_NC = nc
    return _CACHED_NC


def run_sharded(x, positive, negative, **spmd_kwargs):
    """Run the SPMD program; returns BassKernelResults."""
    nc = _get_program()
    x = np.ascontiguousarray(x, dtype=np.float32)
    positive = np.ascontiguousarray(positive, dtype=np.float32)
    negative = np.ascontiguousarray(negative, dtype=np.float32)
    in_maps = [
        {
            "x_shard": x[i * BN_LOC : (i + 1) * BN_LOC],
            "pos_shard": positive[i * BN_LOC : (i + 1) * BN_LOC],
            "negative": negative,
        }
        for i in range(NCORES)
    ]
    res = run_bass_kernel_spmd(nc, in_maps, list(range(NCORES)), **spmd_kwargs)
    return res


def kernel(x, positive, negative):
    res = run_sharded(x, positive, negative)
    total = 0.0
    for i in range(NCORES):
        total += res.results[i]["rowloss"].astype(np.float64).sum()
    return np.asarray(-(total / BN), dtype=np.float32)
